# revision 5
# baseline (speedup 1.0000x reference)
"""KMaxPooling (top-8 along seq axis) Bass kernel for TRN2, 8-core SPMD.

Input  x: (64, 4096, 256) fp32. Output: (64, 8, 256) fp32 = per (batch,
channel) the 8 largest values over the 4096 seq positions, descending.

Strategy (per core, batch-sharded 8 ways -> 8 batches/core, 32 MB):
  - 1 MB quarter-batch DMAs, "(p t) c" split so each partition row is one
    8 KB contiguous DRAM run (128 descriptors/MB instead of 1024 -> less
    per-packet overhead and 8x lower queue dispatch rate)
  - PE transposes 128x128 blocks into PSUM so channels land on partitions
  - DVE InstMax (hardware top-8, sorted desc) over 2048-wide PSUM spans
  - tiny second-level InstMax merges candidates per batch
  - per-batch 8 KB output DMAs overlap the tail; host reassembles layout
"""

import sys

sys.path.insert(0, "/opt/trn_rl_repo")

import numpy as np

import concourse.bass as bass
import concourse.mybir as mybir
from concourse import masks
from concourse.tile import TileContext
from concourse.vector_clock import ScopedClock, VectorClock
from concourse.bass_utils import run_bass_kernel_spmd

B, S, C, K = 64, 4096, 256, 8
NCORES = 8
BPC = B // NCORES  # batches per core
SEQ_TILES = S // 128  # 32
CH_GROUPS = C // 128  # 2
HALF_TILES = SEQ_TILES // 2  # 16 seq tiles per PSUM fill (4 banks)

F32 = mybir.dt.float32

N_PROCS = 27


class SplitDrainTileContext(TileContext):
    """The walrus backend here rejects any instruction carrying more than
    one sync wait ("Too many sync wait commands"), but Tile's semaphore
    assignment can attach several. Two fixes:

    1. _lower_ordered_insts: before lowering, hoist excess waits of every
       scheduled instruction onto single-wait same-engine NoOps inserted
       right before it.
    2. _drain_and_barrier: emit one single-wait drain per logical proc
       instead of one drain waiting on the whole global vector clock.
    """

    def _lower_ordered_insts(self, ordered):
        for bb_name, insts in ordered.items():
            rewritten = []
            for inst in insts:
                si = inst.sync_info
                if si is not None and si.on_wait and len(si.on_wait) > 1:
                    waits = list(si.on_wait)
                    for k, w in enumerate(waits[:-1]):
                        nop = mybir.InstNoOp(
                            name=f"{inst.name}.wsplit{k}",
                            engine=inst.engine,
                            sync_info=mybir.SyncInfo(on_wait=[w], on_update=[]),
                            bass_nofuse=True,
                        )
                        rewritten.append(nop)
                    si.on_wait = waits[-1:]
                rewritten.append(inst)
            ordered[bb_name] = rewritten
        return super()._lower_ordered_insts(ordered)

    def _drain_and_barrier(self, tick_clock, wait_clock):
        gc = tick_clock.global_clock
        for p in range(N_PROCS):
            if gc[p] > 0:
                v = [0] * N_PROCS
                v[p] = gc[p]
                di = self.nc.sync.drain()
                wait_clock.add_sem_waits(di.ins, ScopedClock({None: VectorClock(v)}))

        self.nc.all_engine_barrier()
        assert self.sems is not None
        popped = self.nc._tile_sem_poison_stack.pop()
        assert popped is self._sem_poison
        self.nc.clear_and_free_semaphores(list(self.sems.allocated().values()))
        self.nc.all_engine_barrier()


def build_program():
    nc = bass.Bass()
    x_ext = nc.declare_dram_parameter("x", [BPC, S, C], F32, isOutput=False)
    # out[c', (b*2 + g)*8 + k]: top-k values of channel g*128+c' in batch b
    out_ext = nc.declare_dram_parameter(
        "out", [128, BPC * CH_GROUPS * K], F32, isOutput=True
    )

    QT = HALF_TILES // 2  # 8 seq tiles per quarter load

    with SplitDrainTileContext(nc) as tc:
        with (
            tc.tile_pool(name="const", bufs=1) as const_pool,
            tc.tile_pool(name="xin", bufs=8) as in_pool,
            tc.tile_pool(name="cand", bufs=4) as cand_pool,
            tc.tile_pool(name="obuf", bufs=1) as out_pool,
        ):
            identity = const_pool.tile([128, 128], F32)
            masks.make_identity(nc, identity[:])

            obuf = out_pool.tile([128, BPC * CH_GROUPS * K], F32)

            dma_engines = [nc.sync, nc.scalar]
            dma_idx = [0]

            def load_quarter(b, q):
                """One 1 MB DMA: xin[p, t*C + c] = x[b, q*1024 + p*8 + t, c]
                -> 8 KB contiguous DRAM run per partition."""
                xin = in_pool.tile([128, QT * C], F32)
                seq_lo = q * QT * 128
                dma_engines[dma_idx[0] % 2].dma_start(
                    out=xin[:],
                    in_=x_ext[b, seq_lo : seq_lo + QT * 128].rearrange(
                        "(p t) c -> p t c", p=128
                    ),
                )
                dma_idx[0] += 1
                return xin

            def transpose(ps, ps_col, xin, g, js):
                for j in js:
                    col = j * C + g * 128
                    nc.tensor.matmul(
                        ps[:, ps_col + 128 * j : ps_col + 128 * (j + 1)],
                        xin[:, col : col + 128],
                        identity[:],
                        is_transpose=True,
                        start=True,
                        stop=True,
                    )

            def merge_out(b, g, cand):
                nc.vector.max(
                    out=obuf[:, (b * CH_GROUPS + g) * K : (b * CH_GROUPS + g + 1) * K],
                    in_=cand[:],
                )

            def store_batch(b):
                ocols = CH_GROUPS * K
                dma_engines[dma_idx[0] % 2].dma_start(
                    out=out_ext[:, b * ocols : (b + 1) * ocols],
                    in_=obuf[:, b * ocols : (b + 1) * ocols],
                )
                dma_idx[0] += 1

            cands = {}

            # ---- batches 0..5: sequential 1 MB quarter loads, 4-bank PSUM
            # spans, 2048-wide InstMax per (g, half) -- lowest DVE overhead.
            with tc.tile_pool(name="psumA", bufs=2, space="PSUM") as psum_a:
                pss = {}
                for b in range(BPC - 2):
                    for q in range(4):
                        xin = load_quarter(b, q)
                        h, hq = q // 2, q % 2
                        for g in range(CH_GROUPS):
                            if q == 0:
                                cands[(b, g)] = cand_pool.tile(
                                    [128, 2 * K], F32, name="cand", tag="cand"
                                )
                            if hq == 0:
                                pss[(b, g)] = psum_a.tile(
                                    [128, HALF_TILES * 128], F32, name="ps", tag="ps"
                                )
                            ps = pss[(b, g)]
                            transpose(ps, 1024 * hq, xin, g, range(QT))
                            if hq == 1:
                                nc.vector.max(
                                    out=cands[(b, g)][:, K * h : K * (h + 1)],
                                    in_=ps[:],
                                )
                            if q == 3:
                                merge_out(b, g, cands[(b, g)])
                    store_batch(b)

            # ---- last two batches: loads interleaved (b6q0, b7q0, b6q1, ...)
            # so their DVE work overlaps the stream instead of stacking after
            # it. 2-bank PSUM spans (4 in flight), 1024-wide InstMax per
            # quarter. b7's final quarter is split into 2x 512 KB DMAs with
            # 512-wide InstMax so the post-stream chain is minimal.
            b6, b7 = BPC - 2, BPC - 1
            with tc.tile_pool(name="psumB", bufs=4, space="PSUM") as psum_b:
                for g in range(CH_GROUPS):
                    cands[(b6, g)] = cand_pool.tile(
                        [128, 4 * K], F32, name="cand", tag="cand"
                    )
                    cands[(b7, g)] = cand_pool.tile(
                        [128, 5 * K], F32, name="cand", tag="cand"
                    )
                for q in range(4):
                    # b6 quarter: one 1 MB load, per-quarter 1024-wide InstMax
                    xin = load_quarter(b6, q)
                    for g in range(CH_GROUPS):
                        ps = psum_b.tile([128, QT * 128], F32, name="ps", tag="ps")
                        transpose(ps, 0, xin, g, range(QT))
                        nc.vector.max(
                            out=cands[(b6, g)][:, K * q : K * (q + 1)], in_=ps[:]
                        )
                        if q == 3:
                            merge_out(b6, g, cands[(b6, g)])
                    if q == 3:
                        store_batch(b6)

                    if q < 3:
                        # b7 quarter: same shape as b6
                        xin = load_quarter(b7, q)
                        for g in range(CH_GROUPS):
                            ps = psum_b.tile(
                                [128, QT * 128], F32, name="ps", tag="ps"
                            )
                            transpose(ps, 0, xin, g, range(QT))
                            nc.vector.max(
                                out=cands[(b7, g)][:, K * q : K * (q + 1)],
                                in_=ps[:],
                            )
                    else:
                        # b7 final quarter: 2x 512 KB DMAs (4 KB runs), matmuls
                        # grouped by data half, 512-wide InstMax per (g, half)
                        xin = in_pool.tile([128, QT * C], F32)
                        for hh in range(2):
                            seq_lo = (6 + hh) * QT * 64
                            dma_engines[dma_idx[0] % 2].dma_start(
                                out=xin[
                                    :, hh * QT * C // 2 : (hh + 1) * QT * C // 2
                                ],
                                in_=x_ext[b7, seq_lo : seq_lo + QT * 64].rearrange(
                                    "(p t) c -> p t c", p=128
                                ),
                            )
                            dma_idx[0] += 1
                        pse = {
                            g: psum_b.tile([128, QT * 128], F32, name="ps", tag="ps")
                            for g in range(CH_GROUPS)
                        }
                        for hh in range(2):
                            js = range(hh * 4, (hh + 1) * 4)
                            for g in range(CH_GROUPS):
                                transpose(pse[g], 0, xin, g, js)
                        for g in range(CH_GROUPS):
                            nc.vector.max(
                                out=cands[(b7, g)][:, 3 * K : 4 * K],
                                in_=pse[g][:, 0:512],
                            )
                        for g in range(CH_GROUPS):
                            nc.vector.max(
                                out=cands[(b7, g)][:, 4 * K : 5 * K],
                                in_=pse[g][:, 512:1024],
                            )
                            merge_out(b7, g, cands[(b7, g)])
                        store_batch(b7)

    return nc


_prog = None


def _get_prog():
    global _prog
    if _prog is None:
        _prog = build_program()
    return _prog


def run_on_cores(x: np.ndarray, **run_kwargs):
    """Shard, run on 8 cores, return (full_output, BassKernelResults)."""
    nc = _get_prog()
    x = np.ascontiguousarray(np.asarray(x, dtype=np.float32))
    in_maps = [
        {"x": np.ascontiguousarray(x[i * BPC : (i + 1) * BPC])} for i in range(NCORES)
    ]
    res = run_bass_kernel_spmd(nc, in_maps, list(range(NCORES)), **run_kwargs)
    parts = []
    for i in range(NCORES):
        o = res.results[i]["out"]  # (128, BPC*CH_GROUPS*K)
        o = o.reshape(128, BPC, CH_GROUPS, K)  # (c', b, g, k)
        o = o.transpose(1, 3, 2, 0).reshape(BPC, K, C)  # (b, k, g*128+c')
        parts.append(o)
    return np.concatenate(parts, axis=0), res


def kernel(x: np.ndarray) -> np.ndarray:
    out, _ = run_on_cores(x)
    return out


# revision 6
# speedup vs baseline: 1.0979x; 1.0979x over previous
"""KMaxPooling (top-8 along seq axis) Bass kernel for TRN2, 8-core SPMD.

Input  x: (64, 4096, 256) fp32. Output: (64, 8, 256) fp32 = per (batch,
channel) the 8 largest values over the 4096 seq positions, descending.

Strategy (per core, batch-sharded 8 ways -> 8 batches/core, 32 MB):
  - 1 MB quarter-batch DMAs, "(p t) c" split so each partition row is one
    8 KB contiguous DRAM run (128 descriptors/MB -> ~26 GB/s/engine on all
    16 DMA engines, the measured line rate)
  - PE transposes 128x128 blocks into PSUM so channels land on partitions
  - DVE InstMax (hardware top-8, sorted desc) over 2048-wide PSUM spans
  - tiny second-level InstMax merges candidates per batch
  - per-batch 8 KB output DMAs overlap the tail; host reassembles layout
  - last batch: per-quarter InstMax, final quarter split into 2x 512 KB
    DMAs with half-ordered matmuls + 512-wide InstMax so the post-stream
    serial chain is minimal
"""

import sys

sys.path.insert(0, "/opt/trn_rl_repo")

import numpy as np

import concourse.bass as bass
import concourse.mybir as mybir
from concourse import masks
from concourse.tile import TileContext
from concourse.vector_clock import ScopedClock, VectorClock
from concourse.bass_utils import run_bass_kernel_spmd

B, S, C, K = 64, 4096, 256, 8
NCORES = 8
BPC = B // NCORES  # batches per core
SEQ_TILES = S // 128  # 32
CH_GROUPS = C // 128  # 2
HALF_TILES = SEQ_TILES // 2  # 16 seq tiles per PSUM fill (4 banks)

F32 = mybir.dt.float32

N_PROCS = 27


class SplitDrainTileContext(TileContext):
    """The walrus backend here rejects any instruction carrying more than
    one sync wait ("Too many sync wait commands"), but Tile's semaphore
    assignment can attach several. Two fixes:

    1. _lower_ordered_insts: before lowering, hoist excess waits of every
       scheduled instruction onto single-wait same-engine NoOps inserted
       right before it.
    2. _drain_and_barrier: emit one single-wait drain per logical proc
       instead of one drain waiting on the whole global vector clock.
    """

    def _lower_ordered_insts(self, ordered):
        for bb_name, insts in ordered.items():
            rewritten = []
            for inst in insts:
                si = inst.sync_info
                if si is not None and si.on_wait and len(si.on_wait) > 1:
                    waits = list(si.on_wait)
                    for k, w in enumerate(waits[:-1]):
                        nop = mybir.InstNoOp(
                            name=f"{inst.name}.wsplit{k}",
                            engine=inst.engine,
                            sync_info=mybir.SyncInfo(on_wait=[w], on_update=[]),
                            bass_nofuse=True,
                        )
                        rewritten.append(nop)
                    si.on_wait = waits[-1:]
                rewritten.append(inst)
            ordered[bb_name] = rewritten
        return super()._lower_ordered_insts(ordered)

    def _drain_and_barrier(self, tick_clock, wait_clock):
        gc = tick_clock.global_clock
        for p in range(N_PROCS):
            if gc[p] > 0:
                v = [0] * N_PROCS
                v[p] = gc[p]
                di = self.nc.sync.drain()
                wait_clock.add_sem_waits(di.ins, ScopedClock({None: VectorClock(v)}))

        self.nc.all_engine_barrier()
        assert self.sems is not None
        popped = self.nc._tile_sem_poison_stack.pop()
        assert popped is self._sem_poison
        self.nc.clear_and_free_semaphores(list(self.sems.allocated().values()))
        self.nc.all_engine_barrier()


def build_program():
    nc = bass.Bass()
    x_ext = nc.declare_dram_parameter("x", [BPC, S, C], F32, isOutput=False)
    # out[c', (b*2 + g)*8 + k]: top-k values of channel g*128+c' in batch b
    out_ext = nc.declare_dram_parameter(
        "out", [128, BPC * CH_GROUPS * K], F32, isOutput=True
    )

    with SplitDrainTileContext(nc) as tc:
        with (
            tc.tile_pool(name="const", bufs=1) as const_pool,
            tc.tile_pool(name="xin", bufs=8) as in_pool,
            tc.tile_pool(name="psum", bufs=2, space="PSUM") as psum_pool,
            tc.tile_pool(name="cand", bufs=4) as cand_pool,
            tc.tile_pool(name="obuf", bufs=1) as out_pool,
        ):
            identity = const_pool.tile([128, 128], F32)
            masks.make_identity(nc, identity[:])

            obuf = out_pool.tile([128, BPC * CH_GROUPS * K], F32)

            QT = HALF_TILES // 2  # 8 seq tiles per quarter load
            dma_engines = [nc.sync, nc.scalar]
            cands = {}
            pss = {}
            for b in range(BPC):
                last_b = b == BPC - 1
                for q in range(4):
                    split = last_b and q == 3
                    xin = in_pool.tile([128, QT * C], F32)
                    if not split:
                        # xin[p, t*C + c] = x[b, q*1024 + p*8 + t, c]
                        # -> 8 KB contiguous DRAM run per partition
                        seq_lo = q * QT * 128
                        dma_engines[q % 2].dma_start(
                            out=xin[:],
                            in_=x_ext[b, seq_lo : seq_lo + QT * 128].rearrange(
                                "(p t) c -> p t c", p=128
                            ),
                        )
                    else:
                        # final quarter: two 512 KB DMAs so the second half's
                        # matmuls+InstMax are the only post-stream work
                        for hh in range(2):
                            seq_lo = (q * 2 + hh) * QT * 64
                            dma_engines[hh].dma_start(
                                out=xin[
                                    :, hh * QT * C // 2 : (hh + 1) * QT * C // 2
                                ],
                                in_=x_ext[b, seq_lo : seq_lo + QT * 64].rearrange(
                                    "(p t) c -> p t c", p=128
                                ),
                            )
                    h, hq = q // 2, q % 2
                    for g in range(CH_GROUPS):
                        if q == 0:
                            nslots = 5 * K if last_b else 2 * K
                            cands[(b, g)] = cand_pool.tile(
                                [128, nslots], F32, name="cand", tag="cand"
                            )
                        if hq == 0:
                            pss[(b, g)] = psum_pool.tile(
                                [128, HALF_TILES * 128], F32, name="ps", tag="ps"
                            )
                    if split:
                        # matmuls grouped by 512 KB half: j0-3 (first half)
                        # for both g, then j4-7
                        for hh in range(2):
                            for g in range(CH_GROUPS):
                                for j in range(hh * 4, (hh + 1) * 4):
                                    col = j * C + g * 128
                                    ps = pss[(b, g)]
                                    nc.tensor.matmul(
                                        ps[:, 1024 + 128 * j : 1024 + 128 * (j + 1)],
                                        xin[:, col : col + 128],
                                        identity[:],
                                        is_transpose=True,
                                        start=True,
                                        stop=True,
                                    )
                        # 512-wide InstMax per (g, half): e0 depends only on
                        # the first 512 KB, e1 on the second
                        for g in range(CH_GROUPS):
                            nc.vector.max(
                                out=cands[(b, g)][:, 3 * K : 4 * K],
                                in_=pss[(b, g)][:, 1024:1536],
                            )
                        for g in range(CH_GROUPS):
                            cand = cands[(b, g)]
                            nc.vector.max(
                                out=cand[:, 4 * K : 5 * K],
                                in_=pss[(b, g)][:, 1536:2048],
                            )
                            nc.vector.max(
                                out=obuf[
                                    :,
                                    (b * CH_GROUPS + g) * K : (b * CH_GROUPS + g + 1)
                                    * K,
                                ],
                                in_=cand[:],
                            )
                        dma_engines[b % 2].dma_start(
                            out=out_ext[:, b * CH_GROUPS * K : (b + 1) * CH_GROUPS * K],
                            in_=obuf[:, b * CH_GROUPS * K : (b + 1) * CH_GROUPS * K],
                        )
                        continue
                    for g in range(CH_GROUPS):
                        cand = cands[(b, g)]
                        ps = pss[(b, g)]
                        for j in range(QT):
                            col = j * C + g * 128
                            nc.tensor.matmul(
                                ps[:, 128 * (hq * QT + j) : 128 * (hq * QT + j + 1)],
                                xin[:, col : col + 128],
                                identity[:],
                                is_transpose=True,
                                start=True,
                                stop=True,
                            )
                        if last_b:
                            nc.vector.max(
                                out=cand[:, K * q : K * (q + 1)],
                                in_=ps[:, 1024 * hq : 1024 * (hq + 1)],
                            )
                        elif hq == 1:
                            nc.vector.max(out=cand[:, K * h : K * (h + 1)], in_=ps[:])
                        if q == 3:
                            nc.vector.max(
                                out=obuf[
                                    :,
                                    (b * CH_GROUPS + g) * K : (b * CH_GROUPS + g + 1)
                                    * K,
                                ],
                                in_=cand[:],
                            )
                    if q == 3:
                        # batch done: stream its 16 output columns out now so
                        # the final DMA isn't serialized behind the last merge
                        ocols = CH_GROUPS * K
                        dma_engines[b % 2].dma_start(
                            out=out_ext[:, b * ocols : (b + 1) * ocols],
                            in_=obuf[:, b * ocols : (b + 1) * ocols],
                        )

    return nc


_prog = None


def _get_prog():
    global _prog
    if _prog is None:
        _prog = build_program()
    return _prog


def run_on_cores(x: np.ndarray, **run_kwargs):
    """Shard, run on 8 cores, return (full_output, BassKernelResults)."""
    nc = _get_prog()
    x = np.ascontiguousarray(np.asarray(x, dtype=np.float32))
    in_maps = [
        {"x": np.ascontiguousarray(x[i * BPC : (i + 1) * BPC])} for i in range(NCORES)
    ]
    res = run_bass_kernel_spmd(nc, in_maps, list(range(NCORES)), **run_kwargs)
    parts = []
    for i in range(NCORES):
        o = res.results[i]["out"]  # (128, BPC*CH_GROUPS*K)
        o = o.reshape(128, BPC, CH_GROUPS, K)  # (c', b, g, k)
        o = o.transpose(1, 3, 2, 0).reshape(BPC, K, C)  # (b, k, g*128+c')
        parts.append(o)
    return np.concatenate(parts, axis=0), res


def kernel(x: np.ndarray) -> np.ndarray:
    out, _ = run_on_cores(x)
    return out
